# revision 14
# baseline (speedup 1.0000x reference)
"""DiffusionGraphConv Trainium2 kernel (v2: bf16 end-to-end, fused fin).

Math (per batch b, support s, A = supports[s]):
  x0 = concat(inputs, state)                      # [N, F=128]
  reference out = sum_k x_k @ W_k  (+bias), k in {x0, x1_s0, x2_s0, x1_s1, x2_s1}
  with x1 = A x0, x2 = 2 A A x0 - x0, W_k = weight[f*5+k, :].

Restructured to avoid any on-chip transposes:
  out = x0 @ What + bias + sum_s A_s @ (x0 @ W1_s + A_s @ (x0 @ (2*W2_s)))
  with What = W_0 - W_2 - W_4, (W1_s, W2_s) = (W_1, W_2) for s=0, (W_3, W_4) for s=1.

v2 layout/schedule:
  - all inputs + the output in bf16 (halves DMA bytes; matmul speed is the
    same 1 col/cycle as fp32r, accuracy ~1e-3 vs the 2e-2 gate)
  - sa phase stages [w1p|u] pairs (bf16) and whp = what-part + bias
  - v phase: v_s = A_s @ u_s + w1p_s  (16 banks per support)
  - single fused fin phase: pf = sum_mi A0@v0 + sum_mi A1@v1 (16 chained
    matmuls per bank), out = pf + whp, DMA per bank. No separate fins
    accumulator, no standalone bias pass, no warm-up dummies.
"""

import sys as _sys
import types as _types

try:
    import antenv.axon_hooks  # noqa: F401
except Exception:
    try:
        import antenv as _antenv

        _m = _types.ModuleType("antenv.axon_hooks")
        _m._hook = None
        _m.set_axon_ntff_profile_hook = lambda h: setattr(_m, "_hook", h)
        _m.get_axon_ntff_profile_hook = lambda: _m._hook
        _sys.modules["antenv.axon_hooks"] = _m
        _antenv.axon_hooks = _m
    except Exception:
        pass

import ml_dtypes
import numpy as np

import concourse.mybir as mybir
import concourse.tile as tile
from concourse import bacc
from concourse.bass_utils import run_bass_kernel_spmd

NCORES = 8
B = 64
BL = B // NCORES  # 8 batches per core
N = 1024
F = 128
O = 128
NCH = N // 128  # 8 node chunks

BF16 = mybir.dt.bfloat16
F32 = mybir.dt.float32
NPBF16 = ml_dtypes.bfloat16

_CACHE = {}


def _build():
    if "nc" in _CACHE:
        return _CACHE["nc"]

    nc = bacc.Bacc(trn_type="TRN2", num_devices=NCORES, debug=False)

    x0t_d = nc.dram_tensor("x0t", [BL, F, N], BF16, kind="ExternalInput")
    at_d = nc.dram_tensor("at", [2, N, N], BF16, kind="ExternalInput")
    # host pre-rearranged to [F, 5*O] so the DMA is contiguous per partition
    w_d = nc.dram_tensor("w", [F, 5 * O], BF16, kind="ExternalInput")
    # host pre-tiled bias: 128 identical rows (batch-invariant, reused per b)
    b_d = nc.dram_tensor("b", [128, O], F32, kind="ExternalInput")
    out_d = nc.dram_tensor("out", [N, BL, O], BF16, kind="ExternalOutput")

    with tile.TileContext(nc) as tc:
        with (
            tc.tile_pool(name="big", bufs=1) as big,
            tc.tile_pool(name="small", bufs=1) as small,
            tc.tile_pool(name="ps_pool", bufs=8, space="PSUM") as ps_pool,
        ):
            # ---- persistent tiles ----
            # wc[:, k, :] = W_k; after prep: slot 0 -> What, 2/4 -> 2*W2/2*W4
            wc = small.tile([F, 5, O], BF16)
            bt = small.tile([128, O], F32)  # bias, tiled over partitions
            x0t_t = big.tile([F, BL, N], BF16, name="x0t_t")  # 16KB/part
            at0 = big.tile([128, NCH, N], BF16, name="at0")  # 16KB/part
            at1 = big.tile([128, NCH, N], BF16, name="at1")  # 16KB/part
            # staging pairs per (mi, b): [w1p | u]
            st0 = big.tile([128, NCH, BL, 256], BF16, name="st0")  # 32KB/p
            st1 = big.tile([128, NCH, BL, 256], BF16, name="st1")  # 32KB/p
            whp = big.tile([128, NCH, BL, 128], BF16, name="whp")  # 16KB/p
            v0 = big.tile([128, NCH, N], BF16, name="v0")  # 16KB/part
            v1 = big.tile([128, NCH, N], BF16, name="v1")  # 16KB/part
            outb = big.tile([128, NCH, 2, 512], BF16, name="outb")  # 16KB/p

            # ---- input DMAs across three queues, ordered by demand time:
            # scalar: wc (host-prepped), bias, x0t b0-1 (earliest sa work)
            # sync:   x0t b2-3 (+ output later)
            # gpsimd: at0 (gates v0), x0t b4-7, at1 ----
            nc.scalar.dma_start(wc[:], w_d[:])
            nc.scalar.dma_start(bt[:], b_d[:])
            for b in range(2):
                nc.scalar.dma_start(x0t_t[:, b, :], x0t_d[b])
            for b in range(2, 4):
                nc.sync.dma_start(x0t_t[:, b, :], x0t_d[b])
            for mi in range(NCH):
                nc.gpsimd.dma_start(
                    at0[:, mi, :], at_d[0, mi * 128 : (mi + 1) * 128, :]
                )
            for b in range(4, 8):
                nc.gpsimd.dma_start(x0t_t[:, b, :], x0t_d[b])
            for mi in range(NCH):
                nc.gpsimd.dma_start(
                    at1[:, mi, :], at_d[1, mi * 128 : (mi + 1) * 128, :]
                )

            # ---- HAM warm-up: a few dummy matmuls while the first x0t
            # batch streams in ----
            dummy = small.tile([128, 256], BF16)
            dsink = small.tile([128, 1], F32)
            nc.vector.memset(dummy[:], 0.0)
            for _ in range(4):
                pw = ps_pool.tile([128, 256], F32, name="ps_w", tag="ps")
                nc.tensor.matmul(
                    pw[:], dummy[:, 0:128], dummy[:], start=True, stop=True
                )
            nc.vector.tensor_copy(dsink[:], pw[:, 0:1])

            flip = [0]

            # ---- sa step (s, b, mi): stationary x0T chunk
            #   s=0: stream [What|W1|2*W2]: whp <- what-part + bias, pair -> st0
            #   s=1: stream [W3|2*W4]: pair -> st1
            def sa_step(s, b, mi):
                st = st0 if s == 0 else st1
                wid = 384 if s == 0 else 256
                ps = ps_pool.tile([128, 512], F32, name="ps_sa", tag="ps")
                nc.tensor.matmul(
                    ps[:, :wid],
                    x0t_t[:, b, mi * 128 : (mi + 1) * 128],
                    wc[:, 0:3, :] if s == 0 else wc[:, 3:5, :],
                    start=True,
                    stop=True,
                )
                pair = ps[:, wid - 256 : wid]
                dst = st[:, mi, b, :]
                if s == 0:
                    # ACT takes the pair copy; the bias-fused add must be DVE
                    nc.scalar.copy(dst, pair)
                    nc.vector.tensor_add(whp[:, mi, b, :], ps[:, 0:128], bt[:])
                else:
                    # s1 regions are DVE-light: alternate pair copies
                    flip[0] ^= 1
                    if flip[0]:
                        nc.scalar.copy(dst, pair)
                    else:
                        nc.vector.tensor_copy(dst, pair)

            # ---- v bank (s, ni, b0:b0+nb): v_s = A_s @ u_s + w1p_s over a
            # contiguous batch group (nb=2 lets v0 start after 2 staged b) ----
            def v_bank(s, ni, b0, nb):
                at_t = at0 if s == 0 else at1
                st = st0 if s == 0 else st1
                v = v0 if s == 0 else v1
                w = nb * 128
                pv = ps_pool.tile([128, 512], F32, name="ps_v", tag="ps")
                for mi in range(NCH):
                    nc.tensor.matmul(
                        pv[:, :w],
                        at_t[:, mi, ni * 128 : (ni + 1) * 128],
                        st[:, mi, b0 : b0 + nb, 128:256],
                        start=(mi == 0),
                        stop=(mi == NCH - 1),
                    )
                nc.vector.tensor_add(
                    v[:, ni, b0 * 128 : b0 * 128 + w],
                    pv[:, :w],
                    st[:, ni, b0 : b0 + nb, 0:128],
                )

            # ---- fin bank (ni, h): out = A0@v0 + A1@v1 + whp; DMA out ----
            def fin_bank(ni, h):
                pf = ps_pool.tile([128, 512], F32, name="ps_f", tag="ps")
                for mi in range(NCH):
                    nc.tensor.matmul(
                        pf[:],
                        at0[:, mi, ni * 128 : (ni + 1) * 128],
                        v0[:, mi, h * 512 : (h + 1) * 512],
                        start=(mi == 0),
                        stop=False,
                    )
                for mi in range(NCH):
                    nc.tensor.matmul(
                        pf[:],
                        at1[:, mi, ni * 128 : (ni + 1) * 128],
                        v1[:, mi, h * 512 : (h + 1) * 512],
                        start=False,
                        stop=(mi == NCH - 1),
                    )
                ob = outb[:, ni, h, :]
                nc.vector.tensor_add(ob, pf[:], whp[:, ni, 4 * h : 4 * h + 4, :])
                nc.sync.dma_start(
                    out_d[ni * 128 : (ni + 1) * 128, 4 * h : 4 * h + 4, :], ob
                )

            # ---- schedule (software-pipelined emission) ----
            # head: s0 b0-1 runs while x0t b0-1 (scalar queue) + at0 stream in
            for b in range(2):
                for mi in range(NCH):
                    sa_step(0, b, mi)
            # A: v0 (b0-1, FD256) interleaved with s0 b2-3 staging
            for ni in range(NCH):
                v_bank(0, ni, 0, 2)
                for k in range(2):
                    sa_step(0, 2 + (ni % 2), (ni // 2) * 2 + k)
            # B: v0 (b2-3, FD256) interleaved with s0 b4-7 staging
            for ni in range(NCH):
                v_bank(0, ni, 2, 2)
                for k in range(4):
                    sa_step(0, 4 + (ni // 2), (ni % 2) * 4 + k)
            # C: v0 h1 (b4-7, FD512) interleaved with s1 b0-3 staging
            for ni in range(NCH):
                v_bank(0, ni, 4, 4)
                for k in range(4):
                    sa_step(1, ni // 2, (ni % 2) * 4 + k)
            # D: v1 h0 interleaved with s1 b4-7 staging
            for ni in range(NCH):
                v_bank(1, ni, 0, 4)
                for k in range(4):
                    sa_step(1, 4 + (ni // 2), (ni % 2) * 4 + k)
            # E: v1 h1 (pure PE)
            for ni in range(NCH):
                v_bank(1, ni, 4, 4)
            # fused fin: 16 chained matmuls per bank, one add, one DMA
            for ni in range(NCH):
                for h in range(2):
                    fin_bank(ni, h)

    nc.compile()
    _CACHE["nc"] = nc
    return nc


def kernel(supports, inputs, state, weight, biases, output_size, _trace=False):
    supports = np.asarray(supports, dtype=np.float32)
    inputs = np.asarray(inputs, dtype=np.float32)
    state = np.asarray(state, dtype=np.float32)
    weight = np.asarray(weight, dtype=np.float32)
    biases = np.asarray(biases, dtype=np.float32)
    O_ = int(output_size)
    assert O_ == O and inputs.shape == (B, N * 64) and supports.shape == (2, N, N)

    nc = _build()

    # host staging (layout only): A^T, x0^T, tiled bias row; all bf16
    at_np = np.ascontiguousarray(supports.transpose(0, 2, 1)).astype(NPBF16)
    x0 = np.concatenate(
        [inputs.reshape(B, N, 64), state.reshape(B, N, 64)], axis=2
    )  # [B, N, F]
    x0t = x0.transpose(0, 2, 1)  # [B, F, N] view; per-core slice made contiguous
    # host-side W prep: slots [What, W1, 2*W2, W3, 2*W4] as [F, 5*O] so the
    # SBUF DMA is contiguous and no on-chip weight transform is needed
    wk = weight.reshape(F, 5, O)
    wprep = np.stack(
        [
            wk[:, 0] - wk[:, 2] - wk[:, 4],
            wk[:, 1],
            2.0 * wk[:, 2],
            wk[:, 3],
            2.0 * wk[:, 4],
        ],
        axis=1,
    )
    wb = np.ascontiguousarray(wprep.reshape(F, 5 * O)).astype(NPBF16)
    brow = np.ascontiguousarray(np.broadcast_to(biases[None, :], (128, O))).astype(
        np.float32
    )

    in_maps = []
    for c in range(NCORES):
        in_maps.append(
            {
                "x0t": np.ascontiguousarray(x0t[c * BL : (c + 1) * BL]).astype(NPBF16),
                "at": at_np,
                "w": wb,
                "b": brow,
            }
        )

    res = run_bass_kernel_spmd(
        nc, in_maps, core_ids=list(range(NCORES)), trace=_trace
    )
    kernel.last_result = res

    # out per core: [N, BL, O] bf16 -> full [B, N*O] f32
    parts = [res.results[c]["out"].astype(np.float32) for c in range(NCORES)]
    full = np.concatenate(parts, axis=1)  # [N, B, O]
    return np.ascontiguousarray(full.transpose(1, 0, 2)).reshape(B, N * O_)


# revision 16
# speedup vs baseline: 1.0199x; 1.0199x over previous
"""DiffusionGraphConv Trainium2 kernel (v2: bf16 end-to-end, fused fin).

Math (per batch b, support s, A = supports[s]):
  x0 = concat(inputs, state)                      # [N, F=128]
  reference out = sum_k x_k @ W_k  (+bias), k in {x0, x1_s0, x2_s0, x1_s1, x2_s1}
  with x1 = A x0, x2 = 2 A A x0 - x0, W_k = weight[f*5+k, :].

Restructured to avoid any on-chip transposes:
  out = x0 @ What + bias + sum_s A_s @ (x0 @ W1_s + A_s @ (x0 @ (2*W2_s)))
  with What = W_0 - W_2 - W_4, (W1_s, W2_s) = (W_1, W_2) for s=0, (W_3, W_4) for s=1.

v2 layout/schedule:
  - all inputs + the output in bf16 (halves DMA bytes; matmul speed is the
    same 1 col/cycle as fp32r, accuracy ~1e-3 vs the 2e-2 gate)
  - sa phase stages [w1p|u] pairs (bf16) and whp = what-part + bias
  - v phase: v_s = A_s @ u_s + w1p_s  (16 banks per support)
  - single fused fin phase: pf = sum_mi A0@v0 + sum_mi A1@v1 (16 chained
    matmuls per bank), out = pf + whp, DMA per bank. No separate fins
    accumulator, no standalone bias pass, no warm-up dummies.
"""

import sys as _sys
import types as _types

try:
    import antenv.axon_hooks  # noqa: F401
except Exception:
    try:
        import antenv as _antenv

        _m = _types.ModuleType("antenv.axon_hooks")
        _m._hook = None
        _m.set_axon_ntff_profile_hook = lambda h: setattr(_m, "_hook", h)
        _m.get_axon_ntff_profile_hook = lambda: _m._hook
        _sys.modules["antenv.axon_hooks"] = _m
        _antenv.axon_hooks = _m
    except Exception:
        pass

import ml_dtypes
import numpy as np

import concourse.mybir as mybir
import concourse.tile as tile
from concourse import bacc
from concourse.bass_utils import run_bass_kernel_spmd

NCORES = 8
B = 64
BL = B // NCORES  # 8 batches per core
N = 1024
F = 128
O = 128
NCH = N // 128  # 8 node chunks

BF16 = mybir.dt.bfloat16
F32 = mybir.dt.float32
NPBF16 = ml_dtypes.bfloat16

_CACHE = {}


def _build():
    if "nc" in _CACHE:
        return _CACHE["nc"]

    nc = bacc.Bacc(trn_type="TRN2", num_devices=NCORES, debug=False)

    x0t_d = nc.dram_tensor("x0t", [BL, F, N], BF16, kind="ExternalInput")
    at_d = nc.dram_tensor("at", [2, N, N], BF16, kind="ExternalInput")
    # host pre-rearranged to [F, 5*O] so the DMA is contiguous per partition
    w_d = nc.dram_tensor("w", [F, 5 * O], BF16, kind="ExternalInput")
    # host pre-tiled bias: 128 identical rows (batch-invariant, reused per b)
    b_d = nc.dram_tensor("b", [128, O], F32, kind="ExternalInput")
    out_d = nc.dram_tensor("out", [N, BL, O], BF16, kind="ExternalOutput")

    with tile.TileContext(nc) as tc:
        with (
            tc.tile_pool(name="big", bufs=1) as big,
            tc.tile_pool(name="small", bufs=1) as small,
            tc.tile_pool(name="ps_pool", bufs=8, space="PSUM") as ps_pool,
        ):
            # ---- persistent tiles ----
            # wc[:, k, :] = W_k; after prep: slot 0 -> What, 2/4 -> 2*W2/2*W4
            wc = small.tile([F, 5, O], BF16)
            bt = small.tile([128, O], F32)  # bias, tiled over partitions
            x0t_t = big.tile([F, BL, N], BF16, name="x0t_t")  # 16KB/part
            at0 = big.tile([128, NCH, N], BF16, name="at0")  # 16KB/part
            at1 = big.tile([128, NCH, N], BF16, name="at1")  # 16KB/part
            # staging pairs per (mi, b): [w1p | u]
            st0 = big.tile([128, NCH, BL, 256], BF16, name="st0")  # 32KB/p
            st1 = big.tile([128, NCH, BL, 256], BF16, name="st1")  # 32KB/p
            whp = big.tile([128, NCH, BL, 128], BF16, name="whp")  # 16KB/p
            v0 = big.tile([128, NCH, N], BF16, name="v0")  # 16KB/part
            v1 = big.tile([128, NCH, N], BF16, name="v1")  # 16KB/part
            outb = big.tile([128, NCH, 2, 512], BF16, name="outb")  # 16KB/p

            # ---- input DMAs across three queues, ordered by demand time:
            # scalar: wc (host-prepped), bias, x0t b0-1 (earliest sa work)
            # sync:   x0t b2-3 (+ output later)
            # gpsimd: at0 (gates v0), x0t b4-7, at1 ----
            nc.scalar.dma_start(wc[:], w_d[:])
            nc.scalar.dma_start(bt[:], b_d[:])
            for b in range(4):
                nc.sync.dma_start(x0t_t[:, b, :], x0t_d[b])
            for mi in range(NCH):
                nc.gpsimd.dma_start(
                    at0[:, mi, :], at_d[0, mi * 128 : (mi + 1) * 128, :]
                )
            for b in range(4, 8):
                nc.gpsimd.dma_start(x0t_t[:, b, :], x0t_d[b])
            for mi in range(NCH):
                nc.gpsimd.dma_start(
                    at1[:, mi, :], at_d[1, mi * 128 : (mi + 1) * 128, :]
                )

            # ---- HAM warm-up: a few dummy matmuls while the first x0t
            # batch streams in ----
            dummy = small.tile([128, 256], BF16)
            dsink = small.tile([128, 1], F32)
            nc.vector.memset(dummy[:], 0.0)
            for _ in range(8):
                pw = ps_pool.tile([128, 256], F32, name="ps_w", tag="ps")
                nc.tensor.matmul(
                    pw[:], dummy[:, 0:128], dummy[:], start=True, stop=True
                )
            nc.vector.tensor_copy(dsink[:], pw[:, 0:1])

            flip = [0]

            # ---- sa step (s, b, mi): stationary x0T chunk
            #   s=0: stream [What|W1|2*W2]: whp <- what-part + bias, pair -> st0
            #   s=1: stream [W3|2*W4]: pair -> st1
            def sa_step(s, b, mi):
                st = st0 if s == 0 else st1
                wid = 384 if s == 0 else 256
                ps = ps_pool.tile([128, 512], F32, name="ps_sa", tag="ps")
                nc.tensor.matmul(
                    ps[:, :wid],
                    x0t_t[:, b, mi * 128 : (mi + 1) * 128],
                    wc[:, 0:3, :] if s == 0 else wc[:, 3:5, :],
                    start=True,
                    stop=True,
                )
                pair = ps[:, wid - 256 : wid]
                dst = st[:, mi, b, :]
                if s == 0:
                    # ACT takes the pair copy; the bias-fused add must be DVE
                    nc.scalar.copy(dst, pair)
                    nc.vector.tensor_add(whp[:, mi, b, :], ps[:, 0:128], bt[:])
                else:
                    # s1 regions are DVE-light: alternate pair copies
                    flip[0] ^= 1
                    if flip[0]:
                        nc.scalar.copy(dst, pair)
                    else:
                        nc.vector.tensor_copy(dst, pair)

            # ---- v bank (s, ni, b0:b0+nb): v_s = A_s @ u_s + w1p_s over a
            # contiguous batch group (nb=2 lets v0 start after 2 staged b) ----
            def v_bank(s, ni, b0, nb):
                at_t = at0 if s == 0 else at1
                st = st0 if s == 0 else st1
                v = v0 if s == 0 else v1
                w = nb * 128
                pv = ps_pool.tile([128, 512], F32, name="ps_v", tag="ps")
                for mi in range(NCH):
                    nc.tensor.matmul(
                        pv[:, :w],
                        at_t[:, mi, ni * 128 : (ni + 1) * 128],
                        st[:, mi, b0 : b0 + nb, 128:256],
                        start=(mi == 0),
                        stop=(mi == NCH - 1),
                    )
                nc.vector.tensor_add(
                    v[:, ni, b0 * 128 : b0 * 128 + w],
                    pv[:, :w],
                    st[:, ni, b0 : b0 + nb, 0:128],
                )

            # ---- fin bank (ni, h): out = A0@v0 + A1@v1 + whp; DMA out ----
            def fin_bank(ni, h):
                pf = ps_pool.tile([128, 512], F32, name="ps_f", tag="ps")
                for mi in range(NCH):
                    nc.tensor.matmul(
                        pf[:],
                        at0[:, mi, ni * 128 : (ni + 1) * 128],
                        v0[:, mi, h * 512 : (h + 1) * 512],
                        start=(mi == 0),
                        stop=False,
                    )
                for mi in range(NCH):
                    nc.tensor.matmul(
                        pf[:],
                        at1[:, mi, ni * 128 : (ni + 1) * 128],
                        v1[:, mi, h * 512 : (h + 1) * 512],
                        start=False,
                        stop=(mi == NCH - 1),
                    )
                ob = outb[:, ni, h, :]
                nc.vector.tensor_add(ob, pf[:], whp[:, ni, 4 * h : 4 * h + 4, :])
                nc.sync.dma_start(
                    out_d[ni * 128 : (ni + 1) * 128, 4 * h : 4 * h + 4, :], ob
                )

            # ---- schedule (software-pipelined emission) ----
            # head: s0 b0-1 runs while x0t b0-1 (scalar queue) + at0 stream in
            for b in range(2):
                for mi in range(NCH):
                    sa_step(0, b, mi)
            # A: v0 (b0-1, FD256) interleaved with s0 b2-3 staging
            for ni in range(NCH):
                v_bank(0, ni, 0, 2)
                for k in range(2):
                    sa_step(0, 2 + (ni % 2), (ni // 2) * 2 + k)
            # B: v0 (b2-3, FD256) interleaved with s0 b4-7 staging
            for ni in range(NCH):
                v_bank(0, ni, 2, 2)
                for k in range(4):
                    sa_step(0, 4 + (ni // 2), (ni % 2) * 4 + k)
            # C: v0 h1 (b4-7, FD512) interleaved with s1 b0-3 staging
            for ni in range(NCH):
                v_bank(0, ni, 4, 4)
                for k in range(4):
                    sa_step(1, ni // 2, (ni % 2) * 4 + k)
            # D: v1 h0 interleaved with s1 b4-7 staging
            for ni in range(NCH):
                v_bank(1, ni, 0, 4)
                for k in range(4):
                    sa_step(1, 4 + (ni // 2), (ni % 2) * 4 + k)
            # E: v1 h1 (pure PE)
            for ni in range(NCH):
                v_bank(1, ni, 4, 4)
            # fused fin: 16 chained matmuls per bank, one add, one DMA
            for ni in range(NCH):
                for h in range(2):
                    fin_bank(ni, h)

    nc.compile()
    _CACHE["nc"] = nc
    return nc


def kernel(supports, inputs, state, weight, biases, output_size, _trace=False):
    supports = np.asarray(supports, dtype=np.float32)
    inputs = np.asarray(inputs, dtype=np.float32)
    state = np.asarray(state, dtype=np.float32)
    weight = np.asarray(weight, dtype=np.float32)
    biases = np.asarray(biases, dtype=np.float32)
    O_ = int(output_size)
    assert O_ == O and inputs.shape == (B, N * 64) and supports.shape == (2, N, N)

    nc = _build()

    # host staging (layout only): A^T, x0^T, tiled bias row; all bf16
    at_np = np.ascontiguousarray(supports.transpose(0, 2, 1)).astype(NPBF16)
    x0 = np.concatenate(
        [inputs.reshape(B, N, 64), state.reshape(B, N, 64)], axis=2
    )  # [B, N, F]
    x0t = x0.transpose(0, 2, 1)  # [B, F, N] view; per-core slice made contiguous
    # host-side W prep: slots [What, W1, 2*W2, W3, 2*W4] as [F, 5*O] so the
    # SBUF DMA is contiguous and no on-chip weight transform is needed
    wk = weight.reshape(F, 5, O)
    wprep = np.stack(
        [
            wk[:, 0] - wk[:, 2] - wk[:, 4],
            wk[:, 1],
            2.0 * wk[:, 2],
            wk[:, 3],
            2.0 * wk[:, 4],
        ],
        axis=1,
    )
    wb = np.ascontiguousarray(wprep.reshape(F, 5 * O)).astype(NPBF16)
    brow = np.ascontiguousarray(np.broadcast_to(biases[None, :], (128, O))).astype(
        np.float32
    )

    in_maps = []
    for c in range(NCORES):
        in_maps.append(
            {
                "x0t": np.ascontiguousarray(x0t[c * BL : (c + 1) * BL]).astype(NPBF16),
                "at": at_np,
                "w": wb,
                "b": brow,
            }
        )

    res = run_bass_kernel_spmd(
        nc, in_maps, core_ids=list(range(NCORES)), trace=_trace
    )
    kernel.last_result = res

    # out per core: [N, BL, O] bf16 -> full [B, N*O] f32
    parts = [res.results[c]["out"].astype(np.float32) for c in range(NCORES)]
    full = np.concatenate(parts, axis=1)  # [N, B, O]
    return np.ascontiguousarray(full.transpose(1, 0, 2)).reshape(B, N * O_)


# revision 17
# speedup vs baseline: 1.0274x; 1.0073x over previous
"""DiffusionGraphConv Trainium2 kernel (v2: bf16 end-to-end, fused fin).

Math (per batch b, support s, A = supports[s]):
  x0 = concat(inputs, state)                      # [N, F=128]
  reference out = sum_k x_k @ W_k  (+bias), k in {x0, x1_s0, x2_s0, x1_s1, x2_s1}
  with x1 = A x0, x2 = 2 A A x0 - x0, W_k = weight[f*5+k, :].

Restructured to avoid any on-chip transposes:
  out = x0 @ What + bias + sum_s A_s @ (x0 @ W1_s + A_s @ (x0 @ (2*W2_s)))
  with What = W_0 - W_2 - W_4, (W1_s, W2_s) = (W_1, W_2) for s=0, (W_3, W_4) for s=1.

v2 layout/schedule:
  - all inputs + the output in bf16 (halves DMA bytes; matmul speed is the
    same 1 col/cycle as fp32r, accuracy ~1e-3 vs the 2e-2 gate)
  - sa phase stages [w1p|u] pairs (bf16) and whp = what-part + bias
  - v phase: v_s = A_s @ u_s + w1p_s  (16 banks per support)
  - single fused fin phase: pf = sum_mi A0@v0 + sum_mi A1@v1 (16 chained
    matmuls per bank), out = pf + whp, DMA per bank. No separate fins
    accumulator, no standalone bias pass, no warm-up dummies.
"""

import sys as _sys
import types as _types

try:
    import antenv.axon_hooks  # noqa: F401
except Exception:
    try:
        import antenv as _antenv

        _m = _types.ModuleType("antenv.axon_hooks")
        _m._hook = None
        _m.set_axon_ntff_profile_hook = lambda h: setattr(_m, "_hook", h)
        _m.get_axon_ntff_profile_hook = lambda: _m._hook
        _sys.modules["antenv.axon_hooks"] = _m
        _antenv.axon_hooks = _m
    except Exception:
        pass

import ml_dtypes
import numpy as np

import concourse.mybir as mybir
import concourse.tile as tile
from concourse import bacc
from concourse.bass_utils import run_bass_kernel_spmd

NCORES = 8
B = 64
BL = B // NCORES  # 8 batches per core
N = 1024
F = 128
O = 128
NCH = N // 128  # 8 node chunks

BF16 = mybir.dt.bfloat16
F32 = mybir.dt.float32
NPBF16 = ml_dtypes.bfloat16

_CACHE = {}


def _build():
    if "nc" in _CACHE:
        return _CACHE["nc"]

    nc = bacc.Bacc(trn_type="TRN2", num_devices=NCORES, debug=False)

    x0t_d = nc.dram_tensor("x0t", [BL, F, N], BF16, kind="ExternalInput")
    at_d = nc.dram_tensor("at", [2, N, N], BF16, kind="ExternalInput")
    # host pre-rearranged to [F, 5*O] so the DMA is contiguous per partition
    w_d = nc.dram_tensor("w", [F, 5 * O], BF16, kind="ExternalInput")
    # host pre-tiled bias: 128 identical rows (batch-invariant, reused per b)
    b_d = nc.dram_tensor("b", [128, O], F32, kind="ExternalInput")
    out_d = nc.dram_tensor("out", [N, BL, O], BF16, kind="ExternalOutput")

    with tile.TileContext(nc) as tc:
        with (
            tc.tile_pool(name="big", bufs=1) as big,
            tc.tile_pool(name="small", bufs=1) as small,
            tc.tile_pool(name="ps_pool", bufs=8, space="PSUM") as ps_pool,
        ):
            # ---- persistent tiles ----
            # wc[:, k, :] = W_k; after prep: slot 0 -> What, 2/4 -> 2*W2/2*W4
            wc = small.tile([F, 5, O], BF16)
            bt = small.tile([128, O], F32)  # bias, tiled over partitions
            x0t_t = big.tile([F, BL, N], BF16, name="x0t_t")  # 16KB/part
            at0 = big.tile([128, NCH, N], BF16, name="at0")  # 16KB/part
            at1 = big.tile([128, NCH, N], BF16, name="at1")  # 16KB/part
            # staging pairs per (mi, b): [w1p | u]
            st0 = big.tile([128, NCH, BL, 256], BF16, name="st0")  # 32KB/p
            st1 = big.tile([128, NCH, BL, 256], BF16, name="st1")  # 32KB/p
            whp = big.tile([128, NCH, BL, 128], BF16, name="whp")  # 16KB/p
            v0 = big.tile([128, NCH, N], BF16, name="v0")  # 16KB/part
            v1 = big.tile([128, NCH, N], BF16, name="v1")  # 16KB/part
            outb = big.tile([128, NCH, 2, 512], BF16, name="outb")  # 16KB/p

            # ---- input DMAs across three queues, ordered by demand time:
            # scalar: wc (host-prepped), bias, x0t b0-1 (earliest sa work)
            # sync:   x0t b2-3 (+ output later)
            # gpsimd: at0 (gates v0), x0t b4-7, at1 ----
            nc.scalar.dma_start(wc[:], w_d[:])
            nc.scalar.dma_start(bt[:], b_d[:])
            for b in range(4):
                nc.sync.dma_start(x0t_t[:, b, :], x0t_d[b])
            for mi in range(NCH):
                nc.gpsimd.dma_start(
                    at0[:, mi, :], at_d[0, mi * 128 : (mi + 1) * 128, :]
                )
            for b in range(4, 8):
                nc.gpsimd.dma_start(x0t_t[:, b, :], x0t_d[b])
            for mi in range(NCH):
                nc.gpsimd.dma_start(
                    at1[:, mi, :], at_d[1, mi * 128 : (mi + 1) * 128, :]
                )

            # ---- HAM warm-up: a few dummy matmuls while the first x0t
            # batch streams in ----
            dummy = small.tile([128, 256], BF16)
            dsink = small.tile([128, 1], F32)
            nc.vector.memset(dummy[:], 0.0)
            for _ in range(16):
                pw = ps_pool.tile([128, 256], F32, name="ps_w", tag="ps")
                nc.tensor.matmul(
                    pw[:], dummy[:, 0:128], dummy[:], start=True, stop=True
                )
            nc.vector.tensor_copy(dsink[:], pw[:, 0:1])

            flip = [0]

            # ---- sa step (s, b, mi): stationary x0T chunk
            #   s=0: stream [What|W1|2*W2]: whp <- what-part + bias, pair -> st0
            #   s=1: stream [W3|2*W4]: pair -> st1
            def sa_step(s, b, mi):
                st = st0 if s == 0 else st1
                wid = 384 if s == 0 else 256
                ps = ps_pool.tile([128, 512], F32, name="ps_sa", tag="ps")
                nc.tensor.matmul(
                    ps[:, :wid],
                    x0t_t[:, b, mi * 128 : (mi + 1) * 128],
                    wc[:, 0:3, :] if s == 0 else wc[:, 3:5, :],
                    start=True,
                    stop=True,
                )
                pair = ps[:, wid - 256 : wid]
                dst = st[:, mi, b, :]
                if s == 0:
                    # ACT takes the pair copy; the bias-fused add must be DVE
                    nc.scalar.copy(dst, pair)
                    nc.vector.tensor_add(whp[:, mi, b, :], ps[:, 0:128], bt[:])
                else:
                    # s1 regions are DVE-light: alternate pair copies
                    flip[0] ^= 1
                    if flip[0]:
                        nc.scalar.copy(dst, pair)
                    else:
                        nc.vector.tensor_copy(dst, pair)

            # ---- v bank (s, ni, b0:b0+nb): v_s = A_s @ u_s + w1p_s over a
            # contiguous batch group (nb=2 lets v0 start after 2 staged b) ----
            def v_bank(s, ni, b0, nb):
                at_t = at0 if s == 0 else at1
                st = st0 if s == 0 else st1
                v = v0 if s == 0 else v1
                w = nb * 128
                pv = ps_pool.tile([128, 512], F32, name="ps_v", tag="ps")
                for mi in range(NCH):
                    nc.tensor.matmul(
                        pv[:, :w],
                        at_t[:, mi, ni * 128 : (ni + 1) * 128],
                        st[:, mi, b0 : b0 + nb, 128:256],
                        start=(mi == 0),
                        stop=(mi == NCH - 1),
                    )
                nc.vector.tensor_add(
                    v[:, ni, b0 * 128 : b0 * 128 + w],
                    pv[:, :w],
                    st[:, ni, b0 : b0 + nb, 0:128],
                )

            # ---- fin bank (ni, h): out = A0@v0 + A1@v1 + whp; DMA out ----
            def fin_bank(ni, h):
                pf = ps_pool.tile([128, 512], F32, name="ps_f", tag="ps")
                for mi in range(NCH):
                    nc.tensor.matmul(
                        pf[:],
                        at0[:, mi, ni * 128 : (ni + 1) * 128],
                        v0[:, mi, h * 512 : (h + 1) * 512],
                        start=(mi == 0),
                        stop=False,
                    )
                for mi in range(NCH):
                    nc.tensor.matmul(
                        pf[:],
                        at1[:, mi, ni * 128 : (ni + 1) * 128],
                        v1[:, mi, h * 512 : (h + 1) * 512],
                        start=False,
                        stop=(mi == NCH - 1),
                    )
                ob = outb[:, ni, h, :]
                nc.vector.tensor_add(ob, pf[:], whp[:, ni, 4 * h : 4 * h + 4, :])
                nc.sync.dma_start(
                    out_d[ni * 128 : (ni + 1) * 128, 4 * h : 4 * h + 4, :], ob
                )

            # ---- schedule (software-pipelined emission) ----
            # head: s0 b0-1 runs while x0t b0-1 (scalar queue) + at0 stream in
            for b in range(2):
                for mi in range(NCH):
                    sa_step(0, b, mi)
            # A: v0 (b0-1, FD256) interleaved with s0 b2-3 staging
            for ni in range(NCH):
                v_bank(0, ni, 0, 2)
                for k in range(2):
                    sa_step(0, 2 + (ni % 2), (ni // 2) * 2 + k)
            # B: v0 (b2-3, FD256) interleaved with s0 b4-7 staging
            for ni in range(NCH):
                v_bank(0, ni, 2, 2)
                for k in range(4):
                    sa_step(0, 4 + (ni // 2), (ni % 2) * 4 + k)
            # C: v0 h1 (b4-7, FD512) interleaved with s1 b0-3 staging
            for ni in range(NCH):
                v_bank(0, ni, 4, 4)
                for k in range(4):
                    sa_step(1, ni // 2, (ni % 2) * 4 + k)
            # D: v1 h0 interleaved with s1 b4-7 staging
            for ni in range(NCH):
                v_bank(1, ni, 0, 4)
                for k in range(4):
                    sa_step(1, 4 + (ni // 2), (ni % 2) * 4 + k)
            # E: v1 h1 (pure PE)
            for ni in range(NCH):
                v_bank(1, ni, 4, 4)
            # fused fin: 16 chained matmuls per bank, one add, one DMA
            for ni in range(NCH):
                for h in range(2):
                    fin_bank(ni, h)

    nc.compile()
    _CACHE["nc"] = nc
    return nc


def kernel(supports, inputs, state, weight, biases, output_size, _trace=False):
    supports = np.asarray(supports, dtype=np.float32)
    inputs = np.asarray(inputs, dtype=np.float32)
    state = np.asarray(state, dtype=np.float32)
    weight = np.asarray(weight, dtype=np.float32)
    biases = np.asarray(biases, dtype=np.float32)
    O_ = int(output_size)
    assert O_ == O and inputs.shape == (B, N * 64) and supports.shape == (2, N, N)

    nc = _build()

    # host staging (layout only): A^T, x0^T, tiled bias row; all bf16
    at_np = np.ascontiguousarray(supports.transpose(0, 2, 1)).astype(NPBF16)
    x0 = np.concatenate(
        [inputs.reshape(B, N, 64), state.reshape(B, N, 64)], axis=2
    )  # [B, N, F]
    x0t = x0.transpose(0, 2, 1)  # [B, F, N] view; per-core slice made contiguous
    # host-side W prep: slots [What, W1, 2*W2, W3, 2*W4] as [F, 5*O] so the
    # SBUF DMA is contiguous and no on-chip weight transform is needed
    wk = weight.reshape(F, 5, O)
    wprep = np.stack(
        [
            wk[:, 0] - wk[:, 2] - wk[:, 4],
            wk[:, 1],
            2.0 * wk[:, 2],
            wk[:, 3],
            2.0 * wk[:, 4],
        ],
        axis=1,
    )
    wb = np.ascontiguousarray(wprep.reshape(F, 5 * O)).astype(NPBF16)
    brow = np.ascontiguousarray(np.broadcast_to(biases[None, :], (128, O))).astype(
        np.float32
    )

    in_maps = []
    for c in range(NCORES):
        in_maps.append(
            {
                "x0t": np.ascontiguousarray(x0t[c * BL : (c + 1) * BL]).astype(NPBF16),
                "at": at_np,
                "w": wb,
                "b": brow,
            }
        )

    res = run_bass_kernel_spmd(
        nc, in_maps, core_ids=list(range(NCORES)), trace=_trace
    )
    kernel.last_result = res

    # out per core: [N, BL, O] bf16 -> full [B, N*O] f32
    parts = [res.results[c]["out"].astype(np.float32) for c in range(NCORES)]
    full = np.concatenate(parts, axis=1)  # [N, B, O]
    return np.ascontiguousarray(full.transpose(1, 0, 2)).reshape(B, N * O_)
